# revision 46
# baseline (speedup 1.0000x reference)
"""Multi-head attention (B=4, S=2048, D=768, H=12) on 8 TRN2 NeuronCores.

Sharding: core = (batch b, query-half). Host permutes each core's sequence so
its 1024 query rows come FIRST (softmax over keys is permutation-invariant;
RoPE uses the permuted position ids, so this is exact). Each core computes
full-sequence K/V for its batch plus Q for rows 0:1024, then SDPA + o_proj.
Zero collectives; output rows disjoint across cores.

Structure (single fused phase, engines balanced):
 - hs arrives HOST-TRANSPOSED as hsT [D, S]; plain wide DMAs spread across
   queues (no xbar-transpose DMAs, no PE transposes)
 - rope tables built on device from position ids (rank-1 matmul + mod-1 via
   int cast + Sin LUT); Q tables = prefix columns of the K tables
 - K/Q projection in T-layout; per-partition bias folded into the psum
   evict via DVE tensor_scalar_add (no rank-1 bias matmuls); V bias added
   during evict from a partition-broadcast tile
 - projection matmuls paired per weight-load: one LDWEIGHTS feeds 2 MMs
   (two output slices), halving PE weight-load stalls
 - SDPA head-pair-major with q-half sub-loops: scores psum [128,1024]
   (head0|head1 in adjacent banks), ONE exp ACT per (skt, qhalf) covering
   both heads; PV accumulates [65,512] per head with ones-column rowsum
 - e-chunk hp+1 projections + V projections drained into PE gaps during
   SDPA (ScalarE exp stream stays dense)
 - normalization: rowsum -> reciprocal (DVE, single row) ->
   partition_broadcast -> DVE scale reading pv psum directly
"""

from collections import deque
from contextlib import ExitStack

import numpy as np

import concourse.bass as bass
import concourse.bacc as bacc
import concourse.mybir as mybir
import concourse.tile as tile
from concourse.bass import ds, ts
from concourse.bass_utils import run_bass_kernel_spmd

F32 = mybir.dt.float32
BF16 = mybir.dt.bfloat16
I32 = mybir.dt.int32
AF = mybir.ActivationFunctionType

B, S, D, H = 4, 2048, 768, 12
HD = 64
SQ = 1024          # query rows per core (prefix of permuted seq)
DC = D // 128      # 6 d-chunks
ST = S // 128      # 16 key tiles of 128
HP = 6             # head pairs
VW = 784           # Vaug width: 12*65=780 used + pad to 16-multiple
ROPE_BASE = 10000.0
TWO_PI = float(2.0 * np.pi)
N_CORES = 8
LAG = 6


def build_nc():
    nc = bacc.Bacc("TRN2", target_bir_lowering=False, debug=False,
                   num_devices=N_CORES)

    hsT = nc.dram_tensor("hsT", [D, S], BF16, kind="ExternalInput")
    # positions pre-cast to f32 on host: the rope-turns matmul (first PE
    # work) then has no DVE dependency (DVE engine boots ~12us in)
    pos = nc.dram_tensor("pos", [1, S], F32, kind="ExternalInput")
    # weights arrive host-prearranged in SBUF layout [128, (c j)] so the
    # load is one fully-contiguous DMA
    wqT = nc.dram_tensor("wqT", [128, DC * D], BF16, kind="ExternalInput")
    wkT = nc.dram_tensor("wkT", [128, DC * D], BF16, kind="ExternalInput")
    wvT = nc.dram_tensor("wvT", [128, DC * D], BF16, kind="ExternalInput")
    woT = nc.dram_tensor("woT", [128, DC * D], BF16, kind="ExternalInput")
    bqt = nc.dram_tensor("bqt", [128, DC], F32, kind="ExternalInput")
    bkt = nc.dram_tensor("bkt", [128, DC], F32, kind="ExternalInput")
    bv = nc.dram_tensor("bv", [1, D], F32, kind="ExternalInput")
    # bf16 output: halves the evict + store traffic; host upcasts to f32
    out = nc.dram_tensor("out", [SQ, D], BF16, kind="ExternalOutput")

    invf_turns_np = ((1.0 / ROPE_BASE) ** (np.arange(32) / 32.0) / TWO_PI
                     ).astype(np.float32)
    invf_dram = nc.inline_tensor(
        np.tile(invf_turns_np, 4).reshape(1, 128), name="invf_turns")

    with tile.TileContext(nc) as tc:
        _body(nc, tc, hsT, pos, wqT, wkT, wvT, woT, bqt, bkt, bv, out,
              invf_dram)
    nc.compile()
    return nc


def _body(nc, tc, hsT, pos, wqT, wkT, wvT, woT, bqt, bkt, bv, out,
          invf_dram):
  with ExitStack() as ctx:
    # ---- long-lived pools, opened first so they own the low addresses ----
    const = ctx.enter_context(tc.tile_pool(name="const", bufs=1))
    persist = ctx.enter_context(tc.tile_pool(name="persist", bufs=1))
    xt_pool = ctx.enter_context(tc.tile_pool(name="xt", bufs=1))
    wpool = ctx.enter_context(tc.tile_pool(name="w", bufs=1))
    expool = ctx.enter_context(tc.tile_pool(name="ex", bufs=16))
    nrm = ctx.enter_context(tc.tile_pool(name="nrm", bufs=1))
    osb = ctx.enter_context(tc.tile_pool(name="osb", bufs=2))
    shp_pool = ctx.enter_context(tc.tile_pool(name="shift", bufs=1))
    scps = ctx.enter_context(tc.tile_pool(name="scps", bufs=2, space="PSUM"))
    pvps = ctx.enter_context(tc.tile_pool(name="pvps", bufs=1, space="PSUM"))
    prps = ctx.enter_context(tc.tile_pool(name="prps", bufs=2, space="PSUM"))

    # ---- constants / input DMAs ----
    # invf/posi on the fast sync HWDGE ring: they gate the PE-warming
    # rope-turns matmuls (the gpsimd SWDGE ring takes ~8us to start)
    invf = const.tile([1, 128], F32, tag="invf")
    nc.sync.dma_start(invf[:], invf_dram[:])
    posi = const.tile([1, S], F32, tag="posi")
    nc.sync.dma_start(posi[:], pos[:])
    bqt_sb = const.tile([128, DC], F32, tag="bqt")
    bkt_sb = const.tile([128, DC], F32, tag="bkt")
    nc.gpsimd.dma_start(bqt_sb[:], bqt[:])
    nc.gpsimd.dma_start(bkt_sb[:], bkt[:])
    bv_sb = const.tile([1, D], F32, tag="bv_sb")
    nc.gpsimd.dma_start(bv_sb[:], bv[:])
    # V bias broadcast to all partitions (keys) once
    bvb = const.tile([128, D], F32, tag="bvb")
    nc.gpsimd.partition_broadcast(bvb[:], bv_sb[:])

    # whole-weight batched loads (one DMA each); chunk dc at cols
    # [dc*768, (dc+1)*768)
    def load_weight(wT_dram, name, pool, eng):
        w_all = pool.tile([128, DC * D], BF16, tag=f"w_{name}",
                          name=f"w_{name}")
        eng.dma_start(w_all[:], wT_dram[:])
        return w_all

    # weights on the scalar ring (contiguous 1.2MB loads); hidden-state
    # halves on the sync ring so the first projections start ~4us in
    wq_sb = load_weight(wqT, "q", wpool, nc.scalar)
    wk_sb = load_weight(wkT, "k", wpool, nc.scalar)

    xT = [xt_pool.tile([128, S], BF16, tag=f"xT{dc}", name=f"xT{dc}")
          for dc in range(DC)]
    for half in range(2):
        for dc in range(DC):
            ring = nc.sync if half == 0 else nc.scalar
            ring.dma_start(xT[dc][:, ts(half, 1024)],
                           hsT[ds(dc * 128, 128), ts(half, 1024)])

    wv_sb = load_weight(wvT, "v", wpool, nc.sync)

    # ---- persistent activation tensors ----
    QT = [persist.tile([128, SQ], BF16, tag=f"QT{e}", name=f"QT{e}")
          for e in range(DC)]
    KT = [persist.tile([128, S], BF16, tag=f"KT{e}", name=f"KT{e}")
          for e in range(DC)]
    Vaug = [persist.tile([128, VW], BF16, tag=f"Vaug{st}",
                         name=f"Vaug{st}") for st in range(ST)]
    attnT = [persist.tile([128, SQ], BF16, tag=f"attnT{e}",
                          name=f"attnT{e}") for e in range(DC)]
    cosR = persist.tile([128, S], BF16, tag="cosR")
    sinS = persist.tile([128, S], BF16, tag="sinS")

    # ---- rope tables from position ids (k tables; q rows are the prefix) --
    # built at full 128-partition width via the 4x-replicated invf row, so
    # the Sin ACT writes cosR/sinS slices directly (no cross-partition copy)
    with tc.tile_pool(name="rope_sb", bufs=1) as rp:
        turns_sb = rp.tile([128, 1024], F32, tag="turns_sb")

        def emit_turn(ch):
            # rank-1 position-angle matmul, staged to SBUF (frees psum);
            # chunk ch lands in turns_sb half ch%2
            turns = scps.tile([128, 512], F32, tag="sc", name="turns")
            nc.tensor.matmul(turns[:], invf[:], posi[:, ts(ch, 512)],
                             start=True, stop=True)
            nc.vector.tensor_copy(turns_sb[:, ts(ch % 2, 512)], turns[:])

        def rope_turns():
            # first PE work of the kernel (warms the PE while the big
            # weight DMAs stream in)
            emit_turn(0)
            emit_turn(1)

        def rope_tables():
            for ch in range(4):           # 512-col chunks
                if ch >= 2:
                    emit_turn(ch)
                for dst, shift in ((sinS, 0.0), (cosR, 0.25)):
                    tsh = rp.tile([128, 512], F32, tag="tsh")
                    nc.vector.tensor_scalar_add(
                        tsh[:], turns_sb[:, ts(ch % 2, 512)], shift)
                    ti = rp.tile([128, 512], I32, tag="ti")
                    nc.vector.tensor_copy(ti[:], tsh[:])
                    tif = rp.tile([128, 512], F32, tag="tif")
                    nc.vector.tensor_copy(tif[:], ti[:])
                    nc.vector.tensor_sub(tsh[:], tsh[:], tif[:])
                    nc.scalar.activation(dst[:, ts(ch, 512)], tsh[:],
                                         AF.Sin, scale=TWO_PI)
                # rotate-half sign: rows 0-31 and 64-95 carry -sin
                for q in (0, 2):
                    nc.vector.tensor_scalar_mul(
                        sinS[ds(32 * q, 32), ts(ch, 512)],
                        sinS[ds(32 * q, 32), ts(ch, 512)], -1.0)
            # preload the exp table set after the last Sin (avoids table-
            # set thrash), well before the first SDPA exp needs it
            dummy = rp.tile([32, 32], BF16, tag="dummy_exp")
            nc.scalar.activation(dummy[:], sinS[ds(0, 32), ds(0, 32)],
                                 AF.Exp)

        # ---- projection helpers ----
        def kq_slice2(dst, w_sb, b_sb, e, i0, i1):
            # two 512-col T-layout projection slices for e-chunk e; the two
            # matmuls per dc share one weight load
            p0 = prps.tile([128, 512], F32, tag="proj", name="proj_p0")
            p1 = prps.tile([128, 512], F32, tag="proj", name="proj_p1")
            for dc in range(DC):
                w = w_sb[:, ds(dc * D + e * 128, 128)]
                nc.tensor.matmul(p0[:], w, xT[dc][:, ts(i0, 512)],
                                 start=(dc == 0), stop=(dc == DC - 1))
                nc.tensor.matmul(p1[:], w, xT[dc][:, ts(i1, 512)],
                                 start=(dc == 0), stop=(dc == DC - 1))
            # e<=1 slices (prologue + hp0 drains) evict on ScalarE via
            # Identity-with-bias — the DVE is the locally-saturated engine
            # in the opening phase while ScalarE has slack there
            for i, p in ((i0, p0), (i1, p1)):
                if e <= 1:
                    nc.scalar.activation(dst[e][:, ts(i, 512)], p[:],
                                         AF.Identity,
                                         bias=b_sb[:, ds(e, 1)])
                else:
                    nc.vector.tensor_scalar_add(dst[e][:, ts(i, 512)],
                                                p[:], b_sb[:, ds(e, 1)])

        def rope_slice(dst, e, lo, n, eng=None):
            # in-place rope on dst[e][:, lo:lo+n] using cosR/sinS cols;
            # shift DMAs ride the sync ring during SDPA (the scalar ring
            # carries the exp stream), the scalar ring in the prologue
            sh = shp_pool.tile([128, 512], BF16, tag="shift", name="sh")
            for q in range(4):
                src_q = (q // 2) * 2 + (1 - q % 2)  # 0<->32, 64<->96
                (eng or nc.sync).dma_start(sh[ds(32 * q, 32), 0:n],
                                           dst[e][ds(32 * src_q, 32),
                                                  ds(lo, n)])
            tmp = shp_pool.tile([128, 512], BF16, tag="ropetmp",
                                name="ropetmp")
            nc.vector.tensor_mul(tmp[:, 0:n], sh[:, 0:n],
                                 sinS[:, ds(lo, n)])
            nc.vector.tensor_mul(dst[e][:, ds(lo, n)], dst[e][:, ds(lo, n)],
                                 cosR[:, ds(lo, n)])
            nc.vector.tensor_add(dst[e][:, ds(lo, n)], dst[e][:, ds(lo, n)],
                                 tmp[:, 0:n])

        def v_proj_pair(st):
            # both 384-col halves of the V projection for key tile st; the
            # two matmuls per dc share one weight (xT) load
            p0 = prps.tile([128, 512], F32, tag="proj", name="vproj_p0")
            p1 = prps.tile([128, 512], F32, tag="proj", name="vproj_p1")
            for dc in range(DC):
                x = xT[dc][:, ts(st, 128)]
                nc.tensor.matmul(p0[:, 0:384], x,
                                 wv_sb[:, ds(dc * D, 384)],
                                 start=(dc == 0), stop=(dc == DC - 1))
                nc.tensor.matmul(p1[:, 0:384], x,
                                 wv_sb[:, ds(dc * D + 384, 384)],
                                 start=(dc == 0), stop=(dc == DC - 1))
            dst = Vaug[st][:, 0:H * 65].rearrange("p (h x) -> p h x", x=65)
            for nt, p in ((0, p0), (1, p1)):
                nc.vector.tensor_add(
                    dst[:, ds(nt * 6, 6), 0:64],
                    p[:, 0:384].rearrange("p (h hd) -> p h hd", hd=64),
                    bvb[:, ds(nt * 384, 384)].rearrange(
                        "p (h hd) -> p h hd", hd=64))
            va = Vaug[st][:, 0:H * 65].rearrange("p (h x) -> p h x", x=65)
            nc.gpsimd.memset(va[:, :, 64:65], 1.0)
            nc.gpsimd.memset(Vaug[st][:, H * 65:], 0.0)

        # ---- prologue ----
        # rope-turns first (PE warms while weights stream in), then the
        # e=0 projections, whose psum evicts sit near the front of the
        # DVE queue so PSUM buffers recycle fast; the rope-table DVE
        # chain runs behind them, then the rope slices
        rope_turns()
        kq_slice2(QT, wq_sb, bqt_sb, 0, 0, 1)
        kq_slice2(KT, wk_sb, bkt_sb, 0, 0, 1)
        kq_slice2(KT, wk_sb, bkt_sb, 0, 2, 3)
        rope_tables()
        rope_slice(QT, 0, 0, 512, eng=nc.scalar)
        rope_slice(QT, 0, 512, 512, eng=nc.scalar)
        rope_slice(KT, 0, 0, 512, eng=nc.scalar)
        rope_slice(KT, 0, 512, 512, eng=nc.scalar)
        rope_slice(KT, 0, 1024, 512, eng=nc.scalar)
        rope_slice(KT, 0, 1536, 512, eng=nc.scalar)

    def v_all():
        for st in range(4, ST):
            v_proj_pair(st)

    # wo loads late; its pool reuses the (closed) rope scratch range
    wop = ctx.enter_context(tc.tile_pool(name="wop", bufs=1))
    wo_sb = load_weight(woT, "o", wop, nc.gpsimd)

    # ---- deferred work closures (drained during SDPA) ----
    def make_work(hp):
        items = deque()
        if hp == 0:
            # V st 0..3 moved off the pre-SDPA critical path; they drain
            # first so they're done well before hp0-qh0's do_pv sweep
            for st in range(4):
                items.append(lambda st=st: v_proj_pair(st))
        if hp < HP - 1:
            e = hp + 1
            for sg in (0, 2):
                items.append(lambda e=e, sg=sg: kq_slice2(KT, wk_sb, bkt_sb,
                                                          e, sg, sg + 1))
                items.append(lambda e=e, sg=sg: rope_slice(KT, e, sg * 512,
                                                           512))
                items.append(lambda e=e, sg=sg: rope_slice(
                    KT, e, (sg + 1) * 512, 512))
            items.append(lambda e=e: kq_slice2(QT, wq_sb, bqt_sb, e, 0, 1))
            items.append(lambda e=e: rope_slice(QT, e, 0, 512))
            items.append(lambda e=e: rope_slice(QT, e, 512, 512))
        return items

    _ocnt = [0]

    def o_pair(st, force_prps=False):
        # both 384-col halves of the o projection for q tile st; shared
        # weight (attnT) loads; alternate psum pools so pairs pipeline
        # (force_prps while scores still own the scps buffers)
        if force_prps or _ocnt[0] % 2 == 0:
            p0 = prps.tile([128, 512], F32, tag="proj", name="o_p0")
            p1 = prps.tile([128, 512], F32, tag="proj", name="o_p1")
        else:
            p0 = scps.tile([128, 1024], F32, tag="sc", name="o_p2")
            p1 = p0
        _ocnt[0] += 1
        two = p0 is not p1
        # second half lands at col 512 when sharing one [128,1024] tile so
        # each matmul stays within a single PSUM bank
        sl1 = ds(0, 384) if two else ds(512, 384)
        for dc in range(DC):
            a = attnT[dc][:, ts(st, 128)]
            nc.tensor.matmul(p0[:, 0:384], a, wo_sb[:, ds(dc * D, 384)],
                             start=(dc == 0), stop=(dc == DC - 1))
            nc.tensor.matmul(p1[:, sl1], a, wo_sb[:, ds(dc * D + 384, 384)],
                             start=(dc == 0), stop=(dc == DC - 1))
        o = osb.tile([128, 768], BF16, tag="o_out", name="o_out")
        nc.vector.tensor_copy(o[:, 0:384], p0[:, 0:384])
        nc.vector.tensor_copy(o[:, 384:768], p1[:, sl1])
        nc.sync.dma_start(out[ts(st, 128), :], o[:])

    # ---- SDPA ----
    pending_norm = [None]
    for hp in range(HP):
        work = make_work(hp)
        for qh in range(2):
            pv = [pvps.tile([128, 512], F32, tag=f"pv{i}", name=f"pv{i}")
                  for i in range(2)]
            exs = [None] * ST

            def do_pv(k, pv=pv, exs=exs, hp=hp):
                for i in range(2):
                    h = 2 * hp + i
                    nc.tensor.matmul(pv[i][ds(0, 65), :],
                                     Vaug[k][:, ds(h * 65, 65)],
                                     exs[k][:, ds(i * 512, 512)],
                                     start=(k == 0), stop=(k == ST - 1))

            for skt in range(ST):
                sc = scps.tile([128, 1024], F32, tag="sc", name="sc")
                for i in range(2):
                    nc.tensor.matmul(
                        sc[:, ds(i * 512, 512)],
                        KT[hp][ds(64 * i, 64), ts(skt, 128)],
                        QT[hp][ds(64 * i, 64), ds(qh * 512, 512)],
                        start=True, stop=True, tile_position=(64 * i, 0))
                e = expool.tile([128, 1024], BF16, tag="ex", name="expt")
                nc.scalar.activation(e[:], sc[:], AF.Exp, scale=0.125)
                exs[skt] = e
                if skt == 2 and pending_norm[0] is not None:
                    # previous sub-iteration's normalize, emitted here so
                    # its psum reads execute with slack
                    pending_norm[0]()
                    pending_norm[0] = None
                if not (hp == 0 and qh == 0) and skt >= LAG:
                    do_pv(skt - LAG)
                # drain deferred projection work into the PE gap; in the
                # final (drain-less) sub-iteration, qh=0-row o_projs fill
                # the slack once the qh=0 norm (flushed at skt==2) lands
                if work:
                    work.popleft()()
                elif hp == HP - 1 and qh == 1 and skt in (7, 11):
                    o_pair((skt - 7) // 4, force_prps=True)
            if hp == 0 and qh == 0:
                # V projections emitted AFTER the qh0 score/exp stream (so
                # scores outrank them on the PE) and BEFORE its PV, which
                # then executes promptly once Vaug lands
                v_all()
                for k in range(ST):
                    do_pv(k)
            else:
                for k in range(ST - LAG, ST):
                    do_pv(k)


            # normalize (deferred): evacuate psum fast (frees the pv banks
            # for the next sub-iteration), then reciprocal on 128 wide
            # partitions via DMA reshape, broadcast, scale
            def norm_fn(pv=pv, hp=hp, qh=qh):
                for i in range(2):
                    rsrow = nrm.tile([1, 512], F32, tag="rsrow",
                                     name="rsrow")
                    nc.vector.tensor_copy(rsrow[:], pv[i][ds(64, 1), :])
                    pvs = nrm.tile([64, 512], F32, tag="pvs", name="pvs")
                    nc.vector.tensor_copy(pvs[:], pv[i][ds(0, 64), :])
                    c4 = nrm.tile([128, 4], F32, tag="c4", name="c4")
                    nc.sync.dma_start(c4[:], rsrow[:])
                    r4 = nrm.tile([128, 4], F32, tag="r4", name="r4")
                    nc.vector.reciprocal(r4[:], c4[:])
                    recb = nrm.tile([1, 512], F32, tag="recb", name="recb")
                    nc.sync.dma_start(recb[:], r4[:])
                    rbs = nrm.tile([64, 512], F32, tag="rbs", name="rbs")
                    nc.gpsimd.partition_broadcast(rbs[:], recb[:])
                    nc.vector.tensor_mul(
                        attnT[hp][ds(64 * i, 64), ds(qh * 512, 512)],
                        pvs[:], rbs[:])

            if hp == HP - 1 and qh == 1:
                # final norm emitted immediately; the remaining qh=0-row
                # o_projs (independent of it) keep the PE busy while its
                # DVE/DMA/gpsimd chain runs, so the qh=1-row o_projs start
                # promptly
                norm_fn()
                o_pair(2)
                o_pair(3)
            else:
                pending_norm[0] = norm_fn

    # ---- o_proj remainder (qh=1 output rows) ----
    for st in range(4, SQ // 128):
        o_pair(st)


_NC_CACHE = None


def _get_nc():
    global _NC_CACHE
    if _NC_CACHE is None:
        _NC_CACHE = build_nc()
    return _NC_CACHE


def kernel(hidden_states, position_ids, wq, bq, wk, bk, wv, bv, wo,
           _trace=False):
    import ml_dtypes
    bf16 = ml_dtypes.bfloat16

    def arrange_w(w):
        # wT[c*128+p, j] -> [p, c*768+j]: the kernel's SBUF weight layout
        wT = np.asarray(w, np.float32).T.astype(bf16)
        return np.ascontiguousarray(
            wT.reshape(DC, 128, D).transpose(1, 0, 2).reshape(128, DC * D))

    hidden_states = np.asarray(hidden_states, dtype=np.float32).astype(bf16)
    position_ids = np.asarray(position_ids, dtype=np.int32)
    wqT = arrange_w(wq)
    wkT = arrange_w(wk)
    wvT = arrange_w(wv)
    woT = arrange_w(wo)
    bqt = np.ascontiguousarray(
        np.asarray(bq, np.float32).reshape(DC, 128).T)
    bkt = np.ascontiguousarray(
        np.asarray(bk, np.float32).reshape(DC, 128).T)
    bv_r = np.ascontiguousarray(np.asarray(bv, np.float32).reshape(1, D))

    nc = _get_nc()
    in_maps = []
    for core in range(N_CORES):
        b, half = core // 2, core % 2
        if half == 0:
            hsp = hidden_states[b]
            posp = position_ids[b]
        else:
            hsp = np.concatenate([hidden_states[b, SQ:],
                                  hidden_states[b, :SQ]], axis=0)
            posp = np.concatenate([position_ids[b, SQ:],
                                   position_ids[b, :SQ]])
        in_maps.append({
            "hsT": np.ascontiguousarray(hsp.T),
            "pos": np.ascontiguousarray(
                posp.reshape(1, S).astype(np.float32)),
            "wqT": wqT, "wkT": wkT, "wvT": wvT, "woT": woT,
            "bqt": bqt, "bkt": bkt, "bv": bv_r,
        })
    res = run_bass_kernel_spmd(nc, in_maps, list(range(N_CORES)),
                               trace=_trace)
    outp = np.empty((B, S, D), np.float32)
    for core in range(N_CORES):
        b, half = core // 2, core % 2
        outp[b, half * SQ:(half + 1) * SQ] = np.asarray(
            res.results[core]["out"], dtype=np.float32)
    if _trace:
        kernel._last_exec_time_ns = res.exec_time_ns
        kernel._last_results = res
    return outp


# revision 47
# speedup vs baseline: 1.0185x; 1.0185x over previous
"""Multi-head attention (B=4, S=2048, D=768, H=12) on 8 TRN2 NeuronCores.

Sharding: core = (batch b, query-half). Host permutes each core's sequence so
its 1024 query rows come FIRST (softmax over keys is permutation-invariant;
RoPE uses the permuted position ids, so this is exact). Each core computes
full-sequence K/V for its batch plus Q for rows 0:1024, then SDPA + o_proj.
Zero collectives; output rows disjoint across cores.

Structure (single fused phase, engines balanced):
 - hs arrives HOST-TRANSPOSED as hsT [D, S]; plain wide DMAs spread across
   queues (no xbar-transpose DMAs, no PE transposes)
 - rope tables built on device from position ids (rank-1 matmul + mod-1 via
   int cast + Sin LUT); Q tables = prefix columns of the K tables
 - K/Q projection in T-layout; per-partition bias folded into the psum
   evict via DVE tensor_scalar_add (no rank-1 bias matmuls); V bias added
   during evict from a partition-broadcast tile
 - projection matmuls paired per weight-load: one LDWEIGHTS feeds 2 MMs
   (two output slices), halving PE weight-load stalls
 - SDPA head-pair-major with q-half sub-loops: scores psum [128,1024]
   (head0|head1 in adjacent banks), ONE exp ACT per (skt, qhalf) covering
   both heads; PV accumulates [65,512] per head with ones-column rowsum
 - e-chunk hp+1 projections + V projections drained into PE gaps during
   SDPA (ScalarE exp stream stays dense)
 - normalization: rowsum -> reciprocal (DVE, single row) ->
   partition_broadcast -> DVE scale reading pv psum directly
"""

from collections import deque
from contextlib import ExitStack

import numpy as np

import concourse.bass as bass
import concourse.bacc as bacc
import concourse.mybir as mybir
import concourse.tile as tile
from concourse.bass import ds, ts
from concourse.bass_utils import run_bass_kernel_spmd

F32 = mybir.dt.float32
BF16 = mybir.dt.bfloat16
I32 = mybir.dt.int32
AF = mybir.ActivationFunctionType

B, S, D, H = 4, 2048, 768, 12
HD = 64
SQ = 1024          # query rows per core (prefix of permuted seq)
DC = D // 128      # 6 d-chunks
ST = S // 128      # 16 key tiles of 128
HP = 6             # head pairs
VW = 784           # Vaug width: 12*65=780 used + pad to 16-multiple
ROPE_BASE = 10000.0
TWO_PI = float(2.0 * np.pi)
N_CORES = 8
LAG = 6


def build_nc():
    nc = bacc.Bacc("TRN2", target_bir_lowering=False, debug=False,
                   num_devices=N_CORES)

    hsT = nc.dram_tensor("hsT", [D, S], BF16, kind="ExternalInput")
    # positions pre-cast to f32 on host: the rope-turns matmul (first PE
    # work) then has no DVE dependency (DVE engine boots ~12us in)
    pos = nc.dram_tensor("pos", [1, S], F32, kind="ExternalInput")
    # weights arrive host-prearranged in SBUF layout [128, (c j)] so the
    # load is one fully-contiguous DMA
    wqT = nc.dram_tensor("wqT", [128, DC * D], BF16, kind="ExternalInput")
    wkT = nc.dram_tensor("wkT", [128, DC * D], BF16, kind="ExternalInput")
    wvT = nc.dram_tensor("wvT", [128, DC * D], BF16, kind="ExternalInput")
    woT = nc.dram_tensor("woT", [128, DC * D], BF16, kind="ExternalInput")
    bqt = nc.dram_tensor("bqt", [128, DC], F32, kind="ExternalInput")
    bkt = nc.dram_tensor("bkt", [128, DC], F32, kind="ExternalInput")
    bv = nc.dram_tensor("bv", [1, D], F32, kind="ExternalInput")
    # bf16 output: halves the evict + store traffic; host upcasts to f32
    out = nc.dram_tensor("out", [SQ, D], BF16, kind="ExternalOutput")

    invf_turns_np = ((1.0 / ROPE_BASE) ** (np.arange(32) / 32.0) / TWO_PI
                     ).astype(np.float32)
    invf_dram = nc.inline_tensor(
        np.tile(invf_turns_np, 4).reshape(1, 128), name="invf_turns")

    with tile.TileContext(nc) as tc:
        _body(nc, tc, hsT, pos, wqT, wkT, wvT, woT, bqt, bkt, bv, out,
              invf_dram)
    nc.compile()
    return nc


def _body(nc, tc, hsT, pos, wqT, wkT, wvT, woT, bqt, bkt, bv, out,
          invf_dram):
  with ExitStack() as ctx:
    # ---- long-lived pools, opened first so they own the low addresses ----
    const = ctx.enter_context(tc.tile_pool(name="const", bufs=1))
    persist = ctx.enter_context(tc.tile_pool(name="persist", bufs=1))
    xt_pool = ctx.enter_context(tc.tile_pool(name="xt", bufs=1))
    wpool = ctx.enter_context(tc.tile_pool(name="w", bufs=1))
    expool = ctx.enter_context(tc.tile_pool(name="ex", bufs=16))
    nrm = ctx.enter_context(tc.tile_pool(name="nrm", bufs=1))
    osb = ctx.enter_context(tc.tile_pool(name="osb", bufs=2))
    shp_pool = ctx.enter_context(tc.tile_pool(name="shift", bufs=1))
    scps = ctx.enter_context(tc.tile_pool(name="scps", bufs=2, space="PSUM"))
    pvps = ctx.enter_context(tc.tile_pool(name="pvps", bufs=1, space="PSUM"))
    prps = ctx.enter_context(tc.tile_pool(name="prps", bufs=2, space="PSUM"))

    # ---- constants / input DMAs ----
    # invf/posi on the fast sync HWDGE ring: they gate the PE-warming
    # rope-turns matmuls (the gpsimd SWDGE ring takes ~8us to start)
    invf = const.tile([1, 128], F32, tag="invf")
    nc.sync.dma_start(invf[:], invf_dram[:])
    posi = const.tile([1, S], F32, tag="posi")
    nc.sync.dma_start(posi[:], pos[:])
    bqt_sb = const.tile([128, DC], F32, tag="bqt")
    bkt_sb = const.tile([128, DC], F32, tag="bkt")
    nc.gpsimd.dma_start(bqt_sb[:], bqt[:])
    nc.gpsimd.dma_start(bkt_sb[:], bkt[:])
    bv_sb = const.tile([1, D], F32, tag="bv_sb")
    nc.gpsimd.dma_start(bv_sb[:], bv[:])
    # V bias broadcast to all partitions (keys) once
    bvb = const.tile([128, D], F32, tag="bvb")
    nc.gpsimd.partition_broadcast(bvb[:], bv_sb[:])

    # whole-weight batched loads (one DMA each); chunk dc at cols
    # [dc*768, (dc+1)*768)
    def load_weight(wT_dram, name, pool, eng):
        w_all = pool.tile([128, DC * D], BF16, tag=f"w_{name}",
                          name=f"w_{name}")
        eng.dma_start(w_all[:], wT_dram[:])
        return w_all

    # weights on the scalar ring (contiguous 1.2MB loads); hidden-state
    # halves on the sync ring so the first projections start ~4us in
    wq_sb = load_weight(wqT, "q", wpool, nc.scalar)
    wk_sb = load_weight(wkT, "k", wpool, nc.scalar)

    xT = [xt_pool.tile([128, S], BF16, tag=f"xT{dc}", name=f"xT{dc}")
          for dc in range(DC)]
    for half in range(2):
        for dc in range(DC):
            ring = nc.sync if half == 0 else nc.scalar
            ring.dma_start(xT[dc][:, ts(half, 1024)],
                           hsT[ds(dc * 128, 128), ts(half, 1024)])

    wv_sb = load_weight(wvT, "v", wpool, nc.sync)

    # ---- persistent activation tensors ----
    QT = [persist.tile([128, SQ], BF16, tag=f"QT{e}", name=f"QT{e}")
          for e in range(DC)]
    KT = [persist.tile([128, S], BF16, tag=f"KT{e}", name=f"KT{e}")
          for e in range(DC)]
    Vaug = [persist.tile([128, VW], BF16, tag=f"Vaug{st}",
                         name=f"Vaug{st}") for st in range(ST)]
    attnT = [persist.tile([128, SQ], BF16, tag=f"attnT{e}",
                          name=f"attnT{e}") for e in range(DC)]
    cosR = persist.tile([128, S], BF16, tag="cosR")
    sinS = persist.tile([128, S], BF16, tag="sinS")

    # ---- rope tables from position ids (k tables; q rows are the prefix) --
    # built at full 128-partition width via the 4x-replicated invf row, so
    # the Sin ACT writes cosR/sinS slices directly (no cross-partition copy)
    with tc.tile_pool(name="rope_sb", bufs=1) as rp:
        turns_sb = rp.tile([128, 1024], F32, tag="turns_sb")

        def emit_turn(ch):
            # rank-1 position-angle matmul, staged to SBUF (frees psum);
            # chunk ch lands in turns_sb half ch%2
            turns = scps.tile([128, 512], F32, tag="sc", name="turns")
            nc.tensor.matmul(turns[:], invf[:], posi[:, ts(ch, 512)],
                             start=True, stop=True)
            nc.vector.tensor_copy(turns_sb[:, ts(ch % 2, 512)], turns[:])

        def rope_turns():
            # first PE work of the kernel (warms the PE while the big
            # weight DMAs stream in)
            emit_turn(0)
            emit_turn(1)

        def rope_tables():
            for ch in range(4):           # 512-col chunks
                if ch >= 2:
                    emit_turn(ch)
                for dst, shift in ((sinS, 0.0), (cosR, 0.25)):
                    tsh = rp.tile([128, 512], F32, tag="tsh")
                    nc.vector.tensor_scalar_add(
                        tsh[:], turns_sb[:, ts(ch % 2, 512)], shift)
                    ti = rp.tile([128, 512], I32, tag="ti")
                    nc.vector.tensor_copy(ti[:], tsh[:])
                    tif = rp.tile([128, 512], F32, tag="tif")
                    nc.vector.tensor_copy(tif[:], ti[:])
                    nc.vector.tensor_sub(tsh[:], tsh[:], tif[:])
                    nc.scalar.activation(dst[:, ts(ch, 512)], tsh[:],
                                         AF.Sin, scale=TWO_PI)
                # rotate-half sign: rows 0-31 and 64-95 carry -sin
                for q in (0, 2):
                    nc.vector.tensor_scalar_mul(
                        sinS[ds(32 * q, 32), ts(ch, 512)],
                        sinS[ds(32 * q, 32), ts(ch, 512)], -1.0)
            # preload the exp table set after the last Sin (avoids table-
            # set thrash), well before the first SDPA exp needs it
            dummy = rp.tile([32, 32], BF16, tag="dummy_exp")
            nc.scalar.activation(dummy[:], sinS[ds(0, 32), ds(0, 32)],
                                 AF.Exp)

        # ---- projection helpers ----
        def kq_slice2(dst, w_sb, b_sb, e, i0, i1):
            # two 512-col T-layout projection slices for e-chunk e; the two
            # matmuls per dc share one weight load
            p0 = prps.tile([128, 512], F32, tag="proj", name="proj_p0")
            p1 = prps.tile([128, 512], F32, tag="proj", name="proj_p1")
            for dc in range(DC):
                w = w_sb[:, ds(dc * D + e * 128, 128)]
                nc.tensor.matmul(p0[:], w, xT[dc][:, ts(i0, 512)],
                                 start=(dc == 0), stop=(dc == DC - 1))
                nc.tensor.matmul(p1[:], w, xT[dc][:, ts(i1, 512)],
                                 start=(dc == 0), stop=(dc == DC - 1))
            nc.vector.tensor_scalar_add(dst[e][:, ts(i0, 512)], p0[:],
                                        b_sb[:, ds(e, 1)])
            nc.vector.tensor_scalar_add(dst[e][:, ts(i1, 512)], p1[:],
                                        b_sb[:, ds(e, 1)])

        def rope_slice(dst, e, lo, n, eng=None):
            # in-place rope on dst[e][:, lo:lo+n] using cosR/sinS cols;
            # shift DMAs ride the sync ring during SDPA (the scalar ring
            # carries the exp stream), the scalar ring in the prologue
            sh = shp_pool.tile([128, 512], BF16, tag="shift", name="sh")
            for q in range(4):
                src_q = (q // 2) * 2 + (1 - q % 2)  # 0<->32, 64<->96
                (eng or nc.sync).dma_start(sh[ds(32 * q, 32), 0:n],
                                           dst[e][ds(32 * src_q, 32),
                                                  ds(lo, n)])
            tmp = shp_pool.tile([128, 512], BF16, tag="ropetmp",
                                name="ropetmp")
            nc.vector.tensor_mul(tmp[:, 0:n], sh[:, 0:n],
                                 sinS[:, ds(lo, n)])
            nc.vector.tensor_mul(dst[e][:, ds(lo, n)], dst[e][:, ds(lo, n)],
                                 cosR[:, ds(lo, n)])
            nc.vector.tensor_add(dst[e][:, ds(lo, n)], dst[e][:, ds(lo, n)],
                                 tmp[:, 0:n])

        def v_proj_pair(st):
            # both 384-col halves of the V projection for key tile st; the
            # two matmuls per dc share one weight (xT) load
            p0 = prps.tile([128, 512], F32, tag="proj", name="vproj_p0")
            p1 = prps.tile([128, 512], F32, tag="proj", name="vproj_p1")
            for dc in range(DC):
                x = xT[dc][:, ts(st, 128)]
                nc.tensor.matmul(p0[:, 0:384], x,
                                 wv_sb[:, ds(dc * D, 384)],
                                 start=(dc == 0), stop=(dc == DC - 1))
                nc.tensor.matmul(p1[:, 0:384], x,
                                 wv_sb[:, ds(dc * D + 384, 384)],
                                 start=(dc == 0), stop=(dc == DC - 1))
            dst = Vaug[st][:, 0:H * 65].rearrange("p (h x) -> p h x", x=65)
            for nt, p in ((0, p0), (1, p1)):
                nc.vector.tensor_add(
                    dst[:, ds(nt * 6, 6), 0:64],
                    p[:, 0:384].rearrange("p (h hd) -> p h hd", hd=64),
                    bvb[:, ds(nt * 384, 384)].rearrange(
                        "p (h hd) -> p h hd", hd=64))
            va = Vaug[st][:, 0:H * 65].rearrange("p (h x) -> p h x", x=65)
            nc.gpsimd.memset(va[:, :, 64:65], 1.0)
            nc.gpsimd.memset(Vaug[st][:, H * 65:], 0.0)

        # ---- prologue ----
        # rope-turns first (PE warms while weights stream in), then the
        # e=0 projections, whose psum evicts sit near the front of the
        # DVE queue so PSUM buffers recycle fast; the rope-table DVE
        # chain runs behind them, then the rope slices
        rope_turns()
        kq_slice2(QT, wq_sb, bqt_sb, 0, 0, 1)
        kq_slice2(KT, wk_sb, bkt_sb, 0, 0, 1)
        kq_slice2(KT, wk_sb, bkt_sb, 0, 2, 3)
        rope_tables()
        rope_slice(QT, 0, 0, 512, eng=nc.scalar)
        rope_slice(QT, 0, 512, 512, eng=nc.scalar)
        rope_slice(KT, 0, 0, 512, eng=nc.scalar)
        rope_slice(KT, 0, 512, 512, eng=nc.scalar)
        rope_slice(KT, 0, 1024, 512, eng=nc.scalar)
        rope_slice(KT, 0, 1536, 512, eng=nc.scalar)

    def v_all():
        for st in range(4, ST):
            v_proj_pair(st)

    # wo loads late; its pool reuses the (closed) rope scratch range
    wop = ctx.enter_context(tc.tile_pool(name="wop", bufs=1))
    wo_sb = load_weight(woT, "o", wop, nc.gpsimd)

    # ---- deferred work closures (drained during SDPA) ----
    def make_work(hp):
        items = deque()
        if hp == 0:
            # V st 0..3 moved off the pre-SDPA critical path; they drain
            # first so they're done well before hp0-qh0's do_pv sweep
            for st in range(4):
                items.append(lambda st=st: v_proj_pair(st))
        if hp < HP - 1:
            e = hp + 1
            for sg in (0, 2):
                items.append(lambda e=e, sg=sg: kq_slice2(KT, wk_sb, bkt_sb,
                                                          e, sg, sg + 1))
                items.append(lambda e=e, sg=sg: rope_slice(KT, e, sg * 512,
                                                           512))
                items.append(lambda e=e, sg=sg: rope_slice(
                    KT, e, (sg + 1) * 512, 512))
            items.append(lambda e=e: kq_slice2(QT, wq_sb, bqt_sb, e, 0, 1))
            items.append(lambda e=e: rope_slice(QT, e, 0, 512))
            items.append(lambda e=e: rope_slice(QT, e, 512, 512))
        return items

    _ocnt = [0]

    def o_pair(st, force_prps=False):
        # both 384-col halves of the o projection for q tile st; shared
        # weight (attnT) loads; alternate psum pools so pairs pipeline
        # (force_prps while scores still own the scps buffers)
        if force_prps or _ocnt[0] % 2 == 0:
            p0 = prps.tile([128, 512], F32, tag="proj", name="o_p0")
            p1 = prps.tile([128, 512], F32, tag="proj", name="o_p1")
        else:
            p0 = scps.tile([128, 1024], F32, tag="sc", name="o_p2")
            p1 = p0
        _ocnt[0] += 1
        two = p0 is not p1
        # second half lands at col 512 when sharing one [128,1024] tile so
        # each matmul stays within a single PSUM bank
        sl1 = ds(0, 384) if two else ds(512, 384)
        for dc in range(DC):
            a = attnT[dc][:, ts(st, 128)]
            nc.tensor.matmul(p0[:, 0:384], a, wo_sb[:, ds(dc * D, 384)],
                             start=(dc == 0), stop=(dc == DC - 1))
            nc.tensor.matmul(p1[:, sl1], a, wo_sb[:, ds(dc * D + 384, 384)],
                             start=(dc == 0), stop=(dc == DC - 1))
        o = osb.tile([128, 768], BF16, tag="o_out", name="o_out")
        nc.vector.tensor_copy(o[:, 0:384], p0[:, 0:384])
        nc.vector.tensor_copy(o[:, 384:768], p1[:, sl1])
        nc.sync.dma_start(out[ts(st, 128), :], o[:])

    # ---- SDPA ----
    pending_norm = [None]
    for hp in range(HP):
        work = make_work(hp)
        for qh in range(2):
            pv = [pvps.tile([128, 512], F32, tag=f"pv{i}", name=f"pv{i}")
                  for i in range(2)]
            exs = [None] * ST

            def do_pv(k, pv=pv, exs=exs, hp=hp):
                for i in range(2):
                    h = 2 * hp + i
                    nc.tensor.matmul(pv[i][ds(0, 65), :],
                                     Vaug[k][:, ds(h * 65, 65)],
                                     exs[k][:, ds(i * 512, 512)],
                                     start=(k == 0), stop=(k == ST - 1))

            for skt in range(ST):
                sc = scps.tile([128, 1024], F32, tag="sc", name="sc")
                for i in range(2):
                    nc.tensor.matmul(
                        sc[:, ds(i * 512, 512)],
                        KT[hp][ds(64 * i, 64), ts(skt, 128)],
                        QT[hp][ds(64 * i, 64), ds(qh * 512, 512)],
                        start=True, stop=True, tile_position=(64 * i, 0))
                e = expool.tile([128, 1024], BF16, tag="ex", name="expt")
                nc.scalar.activation(e[:], sc[:], AF.Exp, scale=0.125)
                exs[skt] = e
                if skt == 2 and pending_norm[0] is not None:
                    # previous sub-iteration's normalize, emitted here so
                    # its psum reads execute with slack
                    pending_norm[0]()
                    pending_norm[0] = None
                if not (hp == 0 and qh == 0) and skt >= LAG:
                    do_pv(skt - LAG)
                # drain deferred projection work into the PE gap; in the
                # final (drain-less) sub-iteration, qh=0-row o_projs fill
                # the slack once the qh=0 norm (flushed at skt==2) lands
                if work:
                    work.popleft()()
                elif hp == HP - 1 and qh == 1 and skt in (7, 11):
                    o_pair((skt - 7) // 4, force_prps=True)
            if hp == 0 and qh == 0:
                # V projections emitted AFTER the qh0 score/exp stream (so
                # scores outrank them on the PE) and BEFORE its PV, which
                # then executes promptly once Vaug lands
                v_all()
                for k in range(ST):
                    do_pv(k)
            else:
                for k in range(ST - LAG, ST):
                    do_pv(k)


            # normalize (deferred): evacuate psum fast (frees the pv banks
            # for the next sub-iteration), then reciprocal on 128 wide
            # partitions via DMA reshape, broadcast, scale
            def norm_fn(pv=pv, hp=hp, qh=qh):
                for i in range(2):
                    rsrow = nrm.tile([1, 512], F32, tag="rsrow",
                                     name="rsrow")
                    nc.vector.tensor_copy(rsrow[:], pv[i][ds(64, 1), :])
                    pvs = nrm.tile([64, 512], F32, tag="pvs", name="pvs")
                    nc.vector.tensor_copy(pvs[:], pv[i][ds(0, 64), :])
                    c4 = nrm.tile([128, 4], F32, tag="c4", name="c4")
                    nc.sync.dma_start(c4[:], rsrow[:])
                    r4 = nrm.tile([128, 4], F32, tag="r4", name="r4")
                    nc.vector.reciprocal(r4[:], c4[:])
                    recb = nrm.tile([1, 512], F32, tag="recb", name="recb")
                    nc.sync.dma_start(recb[:], r4[:])
                    rbs = nrm.tile([64, 512], F32, tag="rbs", name="rbs")
                    nc.gpsimd.partition_broadcast(rbs[:], recb[:])
                    nc.vector.tensor_mul(
                        attnT[hp][ds(64 * i, 64), ds(qh * 512, 512)],
                        pvs[:], rbs[:])

            if hp == HP - 1 and qh == 1:
                # final norm emitted immediately; the remaining qh=0-row
                # o_projs (independent of it) keep the PE busy while its
                # DVE/DMA/gpsimd chain runs, so the qh=1-row o_projs start
                # promptly
                norm_fn()
                o_pair(2)
                o_pair(3)
            else:
                pending_norm[0] = norm_fn

    # ---- o_proj remainder (qh=1 output rows) ----
    for st in range(4, SQ // 128):
        o_pair(st)


_NC_CACHE = None


def _get_nc():
    global _NC_CACHE
    if _NC_CACHE is None:
        _NC_CACHE = build_nc()
    return _NC_CACHE


def kernel(hidden_states, position_ids, wq, bq, wk, bk, wv, bv, wo,
           _trace=False):
    import ml_dtypes
    bf16 = ml_dtypes.bfloat16

    def arrange_w(w):
        # wT[c*128+p, j] -> [p, c*768+j]: the kernel's SBUF weight layout
        wT = np.asarray(w, np.float32).T.astype(bf16)
        return np.ascontiguousarray(
            wT.reshape(DC, 128, D).transpose(1, 0, 2).reshape(128, DC * D))

    hidden_states = np.asarray(hidden_states, dtype=np.float32).astype(bf16)
    position_ids = np.asarray(position_ids, dtype=np.int32)
    wqT = arrange_w(wq)
    wkT = arrange_w(wk)
    wvT = arrange_w(wv)
    woT = arrange_w(wo)
    bqt = np.ascontiguousarray(
        np.asarray(bq, np.float32).reshape(DC, 128).T)
    bkt = np.ascontiguousarray(
        np.asarray(bk, np.float32).reshape(DC, 128).T)
    bv_r = np.ascontiguousarray(np.asarray(bv, np.float32).reshape(1, D))

    nc = _get_nc()
    in_maps = []
    for core in range(N_CORES):
        b, half = core // 2, core % 2
        if half == 0:
            hsp = hidden_states[b]
            posp = position_ids[b]
        else:
            hsp = np.concatenate([hidden_states[b, SQ:],
                                  hidden_states[b, :SQ]], axis=0)
            posp = np.concatenate([position_ids[b, SQ:],
                                   position_ids[b, :SQ]])
        in_maps.append({
            "hsT": np.ascontiguousarray(hsp.T),
            "pos": np.ascontiguousarray(
                posp.reshape(1, S).astype(np.float32)),
            "wqT": wqT, "wkT": wkT, "wvT": wvT, "woT": woT,
            "bqt": bqt, "bkt": bkt, "bv": bv_r,
        })
    res = run_bass_kernel_spmd(nc, in_maps, list(range(N_CORES)),
                               trace=_trace)
    outp = np.empty((B, S, D), np.float32)
    for core in range(N_CORES):
        b, half = core // 2, core % 2
        outp[b, half * SQ:(half + 1) * SQ] = np.asarray(
            res.results[core]["out"], dtype=np.float32)
    if _trace:
        kernel._last_exec_time_ns = res.exec_time_ns
        kernel._last_results = res
    return outp
